# revision 19
# baseline (speedup 1.0000x reference)
"""BRITS-style RNN imputation kernel for Trainium2 (8 NeuronCores, data-parallel).

Model dims (hardcoded from the problem spec):
  B=256, T=256, C=64, H=512. Per-core batch shard Bl=32.

v2 design (critical-path focused):
  - Feature-major activations [feat, batch] feed the PE as lhsT; gates are
    computed batch-major into a "hybrid" PSUM tile [128=(strip j, batch b),
    512=(gate g in {i,f,o,g}, h_off)] via 4-way column tiling (tile_position).
  - Sigmoid via tanh trick (i,f,o rows pre-scaled 0.5; cell state doubled).
  - The imputation algebra is folded into bulk-precomputed per-(c,b,t)
    tensors:  c_c = P + A1*(Wf@(w1*xhr)) + A2*xhr   where xhr = Whist@(g*h),
    P = m*v + A1*Zb + A2*bh, A1 = w1*alpha, A2 = w1*(1-alpha), w1 = 1-m,
    Zb = Wf@vmw + b_f, vmw = m*v + w1*bh.  This removes all in-loop
    predications/copies; masks and values never appear in the loop.
  - gamma_h stays in-loop (4 small matmuls + exp) but double-buffered one
    step ahead, off the critical path.
  - Gate PSUM accumulation order: Rm (mask gates, available early) -> Rh
    (recurrent) -> Rcc (imputation) so early work overlaps the previous
    step's pointwise.
  - Output c_c transposed via PE and DMA'd in 4-step batches.
  - Pointwise helpers (Bt, e2) run on GpSimd to unload the DVE.
"""

import os
import sys

sys.path.insert(0, "/opt/trn_rl_repo")

import numpy as np
import ml_dtypes

B, T, C, H = 256, 256, 64, 512
NCORES = 8
BL = B // NCORES  # 32 per-core batch
G4 = 4 * H  # 2048

_cache = {}


def _prep_weights(W_ih, W_hh, b_ih, b_hh, W_gh, b_gh, W_gx, b_gx,
                  W_hist, b_hist, W_feat, b_feat, W_comb, b_comb):
    """Host-side constant prep: permute/scale gate weights into the hybrid
    layout, build transposed chunks, masks, bias rows."""
    f32, bf16 = np.float32, ml_dtypes.bfloat16
    # hybrid gate position (j strip, g' in order i,f,o,g, ho) -> torch row
    base = {0: 0, 1: H, 2: 3 * H, 3: 2 * H}  # i,f,o,g -> torch i,f,g,o bases
    rows = np.zeros(G4, dtype=np.int64)
    scale = np.zeros(G4, dtype=np.float32)
    for j in range(4):
        for gp in range(4):
            for ho in range(128):
                pos = 512 * j + 128 * gp + ho
                rows[pos] = base[gp] + 128 * j + ho
                scale[pos] = 0.5 if gp < 3 else 1.0  # tanh-trick on i,f,o
    Wih_p = (W_ih[rows] * scale[:, None]).astype(f32)   # [2048, 128]
    Whh_p = (W_hh[rows] * scale[:, None]).astype(f32)   # [2048, 512]
    bias_p = ((b_ih + b_hh)[rows] * scale).astype(f32)  # [2048]

    out = {}
    # gates h-chunk streams: Rh[j2] [128, 2048] = Whh_p[:, 128*j2+k].T
    for j2 in range(4):
        out[f"Rh{j2}"] = np.ascontiguousarray(
            Whh_p[:, 128 * j2:128 * (j2 + 1)].T).astype(bf16)
    out["Rcc"] = np.ascontiguousarray(Wih_p[:, :C].T).astype(bf16)  # [64,2048]
    Rm = np.zeros((C + 1, G4), dtype=f32)
    Rm[:C] = Wih_p[:, C:].T
    Rm[C] = bias_p
    out["Rm65"] = Rm.astype(bf16)  # [65, 2048]
    # gamma_h path-B chunks with bias(+ln2) row: [65, 128]
    for j2 in range(4):
        w = np.zeros((C + 1, 128), dtype=f32)
        w[:C] = W_gh[128 * j2:128 * (j2 + 1), :].T
        w[C] = b_gh[128 * j2:128 * (j2 + 1)]
        out[f"Wgh{j2}"] = w.astype(bf16)
    # x_h path-B chunks [128, 64] (raw, bias handled in bulk algebra)
    for j2 in range(4):
        out[f"Whist{j2}"] = np.ascontiguousarray(
            W_hist[:, 128 * j2:128 * (j2 + 1)].T).astype(bf16)
    # z_h: masked feat regression + b_f bias row, [65, 64] bf16
    Wf = np.zeros((C + 1, C), dtype=f32)
    Wf[:C] = (W_feat * (1.0 - np.eye(C, dtype=f32))).T
    Wf[C] = b_feat
    out["Wfeat65"] = Wf.astype(bf16)
    # alpha: two K-chunks. x-part [64, 64] bf16; m-part with bias row [65,64]
    out["WcombX"] = np.ascontiguousarray(W_comb[:, :C].T).astype(bf16)
    Wcm = np.zeros((C + 1, C), dtype=f32)
    Wcm[:C] = W_comb[:, C:].T
    Wcm[C] = b_comb
    out["WcombM65"] = Wcm.astype(bf16)
    # gamma_x per-partition scale/bias columns (fp32)
    out["wgx_neg"] = (-np.diag(W_gx)).reshape(C, 1).astype(f32)
    out["bgx_neg"] = (-b_gx).reshape(C, 1).astype(f32)
    out["bh_col"] = b_hist.reshape(C, 1).astype(f32)
    out["nln2"] = np.full((128, 1), -np.log(2.0), dtype=f32)
    out["identf"] = np.eye(128, dtype=f32)
    out["identb"] = np.eye(128, dtype=f32).astype(bf16)
    return out


def _build_nc(Tn):
    import concourse.bass as bass
    import concourse.bacc as bacc
    import concourse.mybir as mybir
    from concourse.tile import TileContext

    dt = mybir.dt
    AF = mybir.ActivationFunctionType
    ALU = mybir.AluOpType

    nc = bacc.Bacc(None, target_bir_lowering=False, debug=False)

    data_in = nc.declare_dram_parameter("data", [BL, Tn, C], dt.float32, isOutput=False)
    out_d = nc.declare_dram_parameter("out", [BL, Tn, C], dt.float32, isOutput=True)
    wspec = [
        ("Rh0", [128, G4], dt.bfloat16), ("Rh1", [128, G4], dt.bfloat16),
        ("Rh2", [128, G4], dt.bfloat16), ("Rh3", [128, G4], dt.bfloat16),
        ("Rcc", [C, G4], dt.bfloat16), ("Rm65", [C + 1, G4], dt.bfloat16),
        ("Wgh0", [C + 1, 128], dt.bfloat16), ("Wgh1", [C + 1, 128], dt.bfloat16),
        ("Wgh2", [C + 1, 128], dt.bfloat16), ("Wgh3", [C + 1, 128], dt.bfloat16),
        ("Whist0", [128, C], dt.bfloat16), ("Whist1", [128, C], dt.bfloat16),
        ("Whist2", [128, C], dt.bfloat16), ("Whist3", [128, C], dt.bfloat16),
        ("Wfeat65", [C + 1, C], dt.bfloat16),
        ("WcombX", [C, C], dt.bfloat16), ("WcombM65", [C + 1, C], dt.bfloat16),
        ("wgx_neg", [C, 1], dt.float32), ("bgx_neg", [C, 1], dt.float32),
        ("bh_col", [C, 1], dt.float32), ("nln2", [128, 1], dt.float32),
        ("identf", [128, 128], dt.float32), ("identb", [128, 128], dt.bfloat16),
    ]
    wdram = {n: nc.declare_dram_parameter(n, s, d, isOutput=False) for n, s, d in wspec}

    import contextlib
    ctx = contextlib.ExitStack()
    sb = {}
    for n, s, d in wspec:
        sb[n] = ctx.enter_context(nc.sbuf_tensor(f"w_{n}", s, d))

    # loop-persistent stores; free dims (b, t)
    d65 = ctx.enter_context(nc.sbuf_tensor("d65", [C + 1, BL, Tn], dt.bfloat16))
    m65 = ctx.enter_context(nc.sbuf_tensor("m65", [C + 1, BL, Tn], dt.bfloat16))
    w1s = ctx.enter_context(nc.sbuf_tensor("w1s", [C, BL, Tn], dt.bfloat16))
    A1s = ctx.enter_context(nc.sbuf_tensor("A1s", [C, BL, Tn], dt.bfloat16))
    A2s = ctx.enter_context(nc.sbuf_tensor("A2s", [C, BL, Tn], dt.bfloat16))
    Pst = ctx.enter_context(nc.sbuf_tensor("Pst", [C, BL, Tn], dt.float32))
    Cst = ctx.enter_context(nc.sbuf_tensor("Cst", [128, 128], dt.bfloat16))  # 2*c
    # bulk scratch
    S2 = ctx.enter_context(nc.sbuf_tensor("S2", [C, BL, Tn], dt.bfloat16))
    S3 = ctx.enter_context(nc.sbuf_tensor("S3", [C + 1, BL, Tn], dt.bfloat16))
    m_u8 = ctx.enter_context(nc.sbuf_tensor("m_u8", [C, BL, Tn], dt.uint8))
    TQ = min(32, Tn)  # timesteps per DMA chunk
    dbm = ctx.enter_context(nc.sbuf_tensor("dbm", [BL, TQ * C], dt.float32))

    flat = "c b t -> c (b t)"
    flat65 = "c b t -> c (b t)"
    nflat = BL * Tn
    nstep = min(512, nflat)

    with TileContext(nc) as tc:
        with (
            tc.tile_pool(name="ps_g", bufs=2, space="PSUM") as ps_g,
            tc.tile_pool(name="ps_gam", bufs=2, space="PSUM") as ps_gam,
            tc.tile_pool(name="ps_ht", bufs=1, space="PSUM") as ps_ht,
            tc.tile_pool(name="ps_small", bufs=1, space="PSUM") as ps_small,
            tc.tile_pool(name="ps_bulk", bufs=1, space="PSUM") as ps_bulk,
            tc.tile_pool(name="sb_loop", bufs=2) as sbl,
            tc.tile_pool(name="sb_stage", bufs=4) as sbs,
        ):
            # ---------------- bulk phase ----------------
            for n, _, _ in wspec:
                nc.sync.dma_start(out=sb[n][:, :], in_=wdram[n][:, :])
            nc.vector.memset(m65[C:C + 1, :, :], 1.0)
            nc.gpsimd.memset(Pst[:, :, :], 0.0)

            # load + transpose data: masks into m65/m_u8, m*v into Pst
            for q in range(Tn // TQ):
                nc.sync.dma_start(
                    out=dbm[:, :],
                    in_=data_in[:, q * TQ:(q + 1) * TQ, :].rearrange("b t c -> b (t c)"))
                for g in range(TQ // 8):  # groups of 8 timesteps
                    pt = ps_bulk.tile([C, 512], dt.float32, tag="big")
                    for k in range(8):
                        nc.tensor.transpose(
                            pt[:, k * BL:(k + 1) * BL],
                            dbm[:, (g * 8 + k) * C:(g * 8 + k + 1) * C],
                            sb["identf"][:BL, :BL])
                    t0 = q * TQ + g * 8
                    scr = sbs.tile([C, 8 * BL], dt.float32, tag="scr")
                    nc.scalar.copy(scr[:, :], pt[:, :8 * BL])
                    sv = scr[:, :].rearrange("c (k b) -> c k b", k=8)
                    m1 = m65[:C, :, t0:t0 + 8].rearrange("c b k -> c k b")
                    m2 = m_u8[:, :, t0:t0 + 8].rearrange("c b k -> c k b")
                    nc.vector.tensor_tensor(m1, sv, sv, ALU.is_equal)
                    nc.vector.tensor_tensor(m2, sv, sv, ALU.is_equal)
                    dv = Pst[:, :, t0:t0 + 8].rearrange("c b k -> c k b")
                    nc.vector.copy_predicated(dv, m2, sv)

            # w1 = 1 - m
            nc.vector.tensor_scalar(w1s[:, :, :].rearrange(flat),
                                    m65[:C, :, :].rearrange(flat), -1.0, 1.0,
                                    ALU.mult, ALU.add)
            # delta scan: a (S2) = w1 shifted by one t (t>=2); r (S3) = 1 except t=0
            nc.vector.tensor_copy(S2[:, :, 2:], w1s[:, :, 1:Tn - 1])
            nc.vector.memset(S2[:, :, :2], 0.0)
            nc.gpsimd.memset(S3[:C, :, :], 1.0)
            nc.gpsimd.memset(S3[:C, :, 0], 0.0)
            nc.vector.tensor_tensor_scan(
                d65[:C, :, :].rearrange(flat), S2[:, :, :].rearrange(flat),
                S3[:C, :, :].rearrange(flat), 0.0, ALU.mult, ALU.add)
            nc.vector.memset(d65[C:C + 1, :, :], 1.0)
            # gamma_x = min(1, exp(-(d*w + b)))  -> S2 (a dead)
            nc.scalar.activation(S2[:, :, :].rearrange(flat),
                                 d65[:C, :, :].rearrange(flat),
                                 AF.Exp, bias=sb["bgx_neg"][:, 0:1],
                                 scale=sb["wgx_neg"][:, 0:1])
            nc.vector.tensor_scalar_min(S2[:, :, :].rearrange(flat),
                                        S2[:, :, :].rearrange(flat), 1.0)
            # alpha = Wcomb @ [gx; m] + b -> evac bf16 into S3[:C] (r dead)
            for n0 in range(0, nflat, nstep):
                pab = ps_bulk.tile([C, 512], dt.float32, tag="big")
                pa = pab[:, :nstep]
                nc.tensor.matmul(pa[:, :], sb["WcombX"][:, :],
                                 S2[:, :, :].rearrange(flat)[:, n0:n0 + nstep],
                                 start=True, stop=False)
                nc.tensor.matmul(pa[:, :], sb["WcombM65"][:, :],
                                 m65[:, :, :].rearrange(flat65)[:, n0:n0 + nstep],
                                 start=False, stop=True)
                nc.scalar.copy(S3[:C, :, :].rearrange(flat)[:, n0:n0 + nstep], pa[:, :])
            # A1 = w1*alpha ; A2 = w1 - A1
            nc.vector.tensor_tensor(A1s[:, :, :].rearrange(flat),
                                    w1s[:, :, :].rearrange(flat),
                                    S3[:C, :, :].rearrange(flat), ALU.mult)
            nc.vector.tensor_tensor(A2s[:, :, :].rearrange(flat),
                                    w1s[:, :, :].rearrange(flat),
                                    A1s[:, :, :].rearrange(flat), ALU.subtract)
            # e1b = w1*bh -> S2 (gx dead)
            nc.scalar.activation(S2[:, :, :].rearrange(flat),
                                 w1s[:, :, :].rearrange(flat), AF.Copy,
                                 scale=sb["bh_col"][:, 0:1])
            # vmw = m*v + w1*bh -> S3[:C] (alpha dead); ones row for bias
            nc.vector.tensor_tensor(S3[:C, :, :].rearrange(flat),
                                    Pst[:, :, :].rearrange(flat),
                                    S2[:, :, :].rearrange(flat), ALU.add)
            nc.vector.memset(S3[C:C + 1, :, :], 1.0)
            # Zb = Wf @ vmw + b_f -> evac bf16 into S2 (e1b dead)
            for n0 in range(0, nflat, nstep):
                pzb = ps_bulk.tile([C, 512], dt.float32, tag="big")
                pz = pzb[:, :nstep]
                nc.tensor.matmul(pz[:, :], sb["Wfeat65"][:, :],
                                 S3[:, :, :].rearrange(flat65)[:, n0:n0 + nstep],
                                 start=True, stop=True)
                nc.scalar.copy(S2[:, :, :].rearrange(flat)[:, n0:n0 + nstep], pz[:, :])
            # c1 = A2*bh -> S3[:C] (vmw dead after Zb)
            nc.scalar.activation(S3[:C, :, :].rearrange(flat),
                                 A2s[:, :, :].rearrange(flat), AF.Copy,
                                 scale=sb["bh_col"][:, 0:1])
            # d1 = A1*Zb -> S2 in place ; d2 = c1 + d1 -> S2 ; P += d2
            nc.vector.tensor_tensor(S2[:, :, :].rearrange(flat),
                                    A1s[:, :, :].rearrange(flat),
                                    S2[:, :, :].rearrange(flat), ALU.mult)
            nc.vector.tensor_tensor(S2[:, :, :].rearrange(flat),
                                    S3[:C, :, :].rearrange(flat),
                                    S2[:, :, :].rearrange(flat), ALU.add)
            nc.vector.tensor_tensor(Pst[:, :, :].rearrange(flat),
                                    Pst[:, :, :].rearrange(flat),
                                    S2[:, :, :].rearrange(flat), ALU.add)

            # ---------------- recurrent loop ----------------
            hgam = sbl.tile([128, 128], dt.bfloat16, tag="hgam")
            nc.vector.memset(hgam[:, :], 0.0)
            nc.vector.memset(Cst[:, :], 0.0)
            egam_cur = None
            pht = None
            pcc = None
            # step-0 gates: Rm first (mask part, available early; bias row inside)
            pg = ps_g.tile([128, 512], dt.float32, tag="g")
            for j in range(4):
                nc.tensor.matmul(pg[32 * j:32 * (j + 1), :], m65[:, :, 0],
                                 sb["Rm65"][:, 512 * j:512 * (j + 1)],
                                 start=True, stop=False, tile_position=(0, 32 * j))
            for t in range(Tn):
                # apply gamma to h (hgam = gamma.T-weighted h in fm layout)
                if t > 0:
                    hgam = sbl.tile([128, 128], dt.bfloat16, tag="hgam")
                    nc.vector.tensor_tensor(hgam[:, :], pht[:, :], egam_cur[:, :],
                                            ALU.mult)
                # xhr = Whist @ h_gamma  [64, 32] (no bias)
                pxz = ps_small.tile([C, 2 * BL], dt.float32, tag="xz")
                pxh = pxz[:, :BL]
                for j2 in range(4):
                    nc.tensor.matmul(pxh, sb[f"Whist{j2}"][:, :],
                                     hgam[:, j2 * BL:(j2 + 1) * BL],
                                     start=(j2 == 0), stop=(j2 == 3))
                # u = w1*xhr ; e1 = A2*xhr (DVE) ; e2 = e1 + P (Pool)
                u = sbl.tile([C, BL], dt.bfloat16, tag="u")
                nc.vector.tensor_tensor(u[:, :], w1s[:, :, t], pxh, ALU.mult)
                e1 = sbl.tile([C, BL], dt.bfloat16, tag="e1")
                nc.vector.tensor_tensor(e1[:, :], A2s[:, :, t], pxh, ALU.mult)
                e2 = sbl.tile([C, BL], dt.float32, tag="e2")
                nc.gpsimd.tensor_tensor(e2[:, :], e1[:, :], Pst[:, :, t], ALU.add)

                # gates: Rh j2=0,1
                for j2 in range(2):
                    for j in range(4):
                        nc.tensor.matmul(pg[32 * j:32 * (j + 1), :],
                                         hgam[:, j2 * BL:(j2 + 1) * BL],
                                         sb[f"Rh{j2}"][:, 512 * j:512 * (j + 1)],
                                         start=False, stop=False,
                                         tile_position=(0, 32 * j))
                # zz = Wf_masked @ u  (no bias)
                pzz = pxz[:, BL:2 * BL]
                nc.tensor.matmul(pzz, sb["Wfeat65"][:C, :], u[:, :],
                                 start=True, stop=True)
                # gates: Rh j2=2,3
                for j2 in range(2, 4):
                    for j in range(4):
                        nc.tensor.matmul(pg[32 * j:32 * (j + 1), :],
                                         hgam[:, j2 * BL:(j2 + 1) * BL],
                                         sb[f"Rh{j2}"][:, 512 * j:512 * (j + 1)],
                                         start=False, stop=False,
                                         tile_position=(0, 32 * j))
                # q = A1*zz ; ccb = q + e2 (bf16)
                qt = sbl.tile([C, BL], dt.bfloat16, tag="q")
                nc.vector.tensor_tensor(qt[:, :], A1s[:, :, t], pzz, ALU.mult)
                ccb = sbl.tile([C, BL], dt.bfloat16, tag="ccb")
                nc.vector.tensor_tensor(ccb[:, :], qt[:, :], e2[:, :], ALU.add)
                # gates: Rcc (last, stop)
                for j in range(4):
                    nc.tensor.matmul(pg[32 * j:32 * (j + 1), :], ccb[:, :],
                                     sb["Rcc"][:, 512 * j:512 * (j + 1)],
                                     start=False, stop=True, tile_position=(0, 32 * j))
                # output c_c -> [b, t, c], batched over 4 steps
                k4 = t % 4
                if k4 == 0:
                    pcc = ps_small.tile([BL, 4 * C], dt.bfloat16, tag="pcc")
                nc.tensor.transpose(pcc[:, k4 * C:(k4 + 1) * C], ccb[:, :],
                                    sb["identb"][:C, :C])
                if k4 == 3:
                    stg = sbs.tile([BL, 4 * C], dt.float32, tag="stg")
                    nc.scalar.copy(stg[:, :], pcc[:, :])
                    nc.sync.dma_start(
                        out=out_d[:, t - 3:t + 1, :].rearrange("b t c -> b (t c)"),
                        in_=stg[:, :])

                # LSTM pointwise (tanh-trick; Cst = 2c)
                tg = sbl.tile([128, 512], dt.bfloat16, tag="tg")
                nc.scalar.activation(tg[:, :], pg[:, :], AF.Tanh)

                # next step's early PE work fills the pointwise-tail PE gap:
                # gamma_h(t+1) matmuls + Rm(t+1) run before the h transpose.
                if t + 1 < Tn:
                    pgam = ps_gam.tile([128, 512], dt.float32, tag="gam")
                    for j2 in range(4):
                        nc.tensor.matmul(pgam[:, j2 * BL:(j2 + 1) * BL],
                                         sb[f"Wgh{j2}"][:, :], d65[:, :, t + 1],
                                         start=True, stop=True)
                    rgam = sbl.tile([128, 128], dt.bfloat16, tag="rgam")
                    nc.scalar.activation(rgam[:, :], pgam[:, :128], AF.Relu)
                    egam_cur = sbl.tile([128, 128], dt.bfloat16, tag="egam")
                    nc.scalar.activation(egam_cur[:, :], rgam[:, :], AF.Exp,
                                         bias=sb["nln2"][:, 0:1], scale=-1.0)
                    pg = ps_g.tile([128, 512], dt.float32, tag="g")
                    for j in range(4):
                        nc.tensor.matmul(pg[32 * j:32 * (j + 1), :], m65[:, :, t + 1],
                                         sb["Rm65"][:, 512 * j:512 * (j + 1)],
                                         start=True, stop=False,
                                         tile_position=(0, 32 * j))

                At = sbl.tile([128, 128], dt.bfloat16, tag="A")
                nc.vector.scalar_tensor_tensor(At[:, :], tg[:, 128:256], 1.0,
                                               Cst[:, :], ALU.add, ALU.mult)
                Bt = sbl.tile([128, 128], dt.bfloat16, tag="Bt")
                nc.vector.scalar_tensor_tensor(Bt[:, :], tg[:, 0:128], 1.0,
                                               tg[:, 384:512], ALU.add, ALU.mult)
                nc.vector.scalar_tensor_tensor(Cst[:, :], At[:, :], 0.5,
                                               Bt[:, :], ALU.mult, ALU.add)
                tcn = sbl.tile([128, 128], dt.bfloat16, tag="tcn")
                nc.scalar.activation(tcn[:, :], Cst[:, :], AF.Tanh, scale=0.5)
                hh = sbl.tile([128, 128], dt.bfloat16, tag="hh")
                nc.vector.scalar_tensor_tensor(hh[:, :], tg[:, 256:384], 1.0,
                                               tcn[:, :], ALU.add, ALU.mult)
                # h hybrid -> fm via PE transpose (evac fused into gamma mult
                # at the top of step t+1)
                pht = ps_ht.tile([128, 128], dt.bfloat16, tag="ht")
                nc.tensor.transpose(pht[:, :], hh[:, :], sb["identb"][:, :])
    ctx.close()
    nc.compile()
    return nc


def kernel(**inputs):
    data = np.asarray(inputs["data"], dtype=np.float32)
    Tn = data.shape[1]
    key = Tn
    if key not in _cache:
        _cache[key] = _build_nc(Tn)
    nc = _cache[key]

    prep = _prep_weights(
        inputs["W_ih"], inputs["W_hh"], inputs["b_ih"], inputs["b_hh"],
        inputs["W_gh"], inputs["b_gh"], inputs["W_gx"], inputs["b_gx"],
        inputs["W_hist"], inputs["b_hist"], inputs["W_feat"], inputs["b_feat"],
        inputs["W_comb"], inputs["b_comb"])
    prep = {k: np.ascontiguousarray(v) for k, v in prep.items()}

    from concourse.bass_utils import run_bass_kernel_spmd
    in_maps = []
    for i in range(NCORES):
        m = dict(prep)
        m["data"] = np.ascontiguousarray(data[i * BL:(i + 1) * BL])
        in_maps.append(m)
    res = run_bass_kernel_spmd(nc, in_maps, list(range(NCORES)))
    outs = [np.asarray(res.results[i]["out"]) for i in range(NCORES)]
    return np.concatenate(outs, axis=0).astype(np.float32)


if __name__ == "__main__":
    import reference
    inp = reference.setup_inputs()
    inp = {k: np.asarray(v) for k, v in inp.items()}
    Tn = int(os.environ.get("TN", "8"))
    inp["data"] = inp["data"][:, :Tn]
    exp = np.asarray(reference.reference(**{k: v for k, v in inp.items()}))
    act = kernel(**inp)
    err = np.abs(act - exp)
    rel = np.linalg.norm((act - exp).ravel()) / np.linalg.norm(exp.ravel())
    print("max abs err:", np.nanmax(err), "rel:", rel)
